# revision 6
# baseline (speedup 1.0000x reference)
"""Trainium2 Bass kernel for nn_CrossAttentionLayer (cross-attention transformer layer).

Strategy: data-parallel over batch (B=8) across the 8 NeuronCores — one batch
element per core, no collectives. All matmuls in fp32r (full PE rate at N>=512,
~tf32 precision). Softmax computed without max-subtraction (scores ~N(0,1));
key mask folded into the transposed-orientation exp as an ACT bias and into the
natural-orientation path as a multiplicative 0/1 mask fused with the
denominator normalization. Softmax denominators come for free from a
ones-column appended to V in the P@V matmul.
"""
import math
from contextlib import ExitStack
from functools import lru_cache

import numpy as np

import concourse.bass as bass
import concourse.bacc as bacc
import concourse.tile as tile
from concourse import mybir
from concourse.masks import make_identity
from concourse.bass_utils import run_bass_kernel_spmd

F32 = mybir.dt.float32
F32R = mybir.dt.float32r
AF = mybir.ActivationFunctionType
OP = mybir.AluOpType

B, Lm, Lp, D, H = 8, 512, 1024, 768, 12
DK = D // H            # 64
NQ = Lm // 128         # 4 query chunks
NK = Lp // 128         # 8 key chunks
ND = D // 128          # 6 d chunks
NHH = H // 2           # 6 head pairs
DH = 4 * D             # 3072
NDD = DH // 128        # 24 hidden chunks
MASK_BIAS = -1.0e5     # exp(x + MASK_BIAS) == 0 exactly in f32


def _build():
    nc = bacc.Bacc("TRN2", target_bir_lowering=False, debug=False)

    mol = nc.dram_tensor("mol", [Lm, D], F32, kind="ExternalInput")
    molT = nc.dram_tensor("molT", [D, Lm], F32R, kind="ExternalInput")
    protT = nc.dram_tensor("protT", [D, Lp], F32R, kind="ExternalInput")
    maskb = nc.dram_tensor("maskb", [128, NK], F32, kind="ExternalInput")
    mask01 = nc.dram_tensor("mask01", [Lp], F32, kind="ExternalInput")
    wq = nc.dram_tensor("wq", [D, D], F32R, kind="ExternalInput")
    wk = nc.dram_tensor("wk", [D, D], F32R, kind="ExternalInput")
    wv = nc.dram_tensor("wv", [D, D], F32R, kind="ExternalInput")
    wo = nc.dram_tensor("wo", [D, D], F32R, kind="ExternalInput")
    w1p = nc.dram_tensor("w1p", [NDD, D, 128], F32R, kind="ExternalInput")
    w2 = nc.dram_tensor("w2", [DH, D], F32R, kind="ExternalInput")
    bo_c = nc.dram_tensor("bo_c", [128, ND], F32, kind="ExternalInput")
    b1_c = nc.dram_tensor("b1_c", [128, NDD], F32, kind="ExternalInput")
    b2_c = nc.dram_tensor("b2_c", [128, ND], F32, kind="ExternalInput")
    g1 = nc.dram_tensor("g1", [D], F32, kind="ExternalInput")
    bb1 = nc.dram_tensor("bb1", [D], F32, kind="ExternalInput")
    g2 = nc.dram_tensor("g2", [D], F32, kind="ExternalInput")
    bb2 = nc.dram_tensor("bb2", [D], F32, kind="ExternalInput")

    out_d = nc.dram_tensor("out", [Lm, D], F32, kind="ExternalOutput")
    attn_d = nc.dram_tensor("attn", [H, Lm, Lp], F32, kind="ExternalOutput")

    def bcast(dram_ap, n):
        return bass.AP(tensor=dram_ap.tensor, offset=0, ap=[[0, 128], [1, n]])

    with tile.TileContext(nc) as tc, ExitStack() as ctx:
        consts = ctx.enter_context(tc.tile_pool(name="consts", bufs=1))
        persist = ctx.enter_context(tc.tile_pool(name="persist", bufs=1))

        ident = consts.tile([128, 128], F32)
        make_identity(nc, ident[:])
        maskb_t = consts.tile([128, NK], F32)
        nc.sync.dma_start(out=maskb_t[:], in_=maskb[:])
        mask_bc = consts.tile([128, Lp], F32)
        nc.sync.dma_start(out=mask_bc[:], in_=bcast(mask01.ap(), Lp))
        eps_t = consts.tile([128, 1], F32)
        nc.vector.memset(eps_t[:], 1e-5)
        one11 = consts.tile([1, 1], F32)
        nc.vector.memset(one11[:], 1.0)
        ones_f = consts.tile([128, 64], F32)
        nc.vector.memset(ones_f[:], 1.0)
        ones_r = consts.tile([128, 64], F32R)
        nc.vector.tensor_copy(ones_r[:], ones_f[:])
        bo_t = consts.tile([128, ND], F32)
        nc.sync.dma_start(out=bo_t[:], in_=bo_c[:])
        b1_t = consts.tile([128, NDD], F32)
        nc.sync.dma_start(out=b1_t[:], in_=b1_c[:])
        b2_t = consts.tile([128, ND], F32)
        nc.sync.dma_start(out=b2_t[:], in_=b2_c[:])

        # ---------------- Phase 1: QKV projections ----------------
        qTp = persist.tile([128, NHH, Lm], F32R)   # Q^T packed 2 heads/chunk
        kTp = persist.tile([128, NHH, Lp], F32R)   # K^T packed
        vhat = persist.tile([128, NK, H, DK + 1], F32R)  # V natural + ones col
        oT = persist.tile([128, ND, Lm], F32R)     # normalized O^T (concat heads)

        with tc.tile_pool(name="ph1", bufs=1) as ph1, \
             tc.tile_pool(name="wslab", bufs=2) as wslab, \
             tc.tile_pool(name="ph1ps", bufs=4, space="PSUM") as ph1ps:
            molT_sb = ph1.tile([128, ND, Lm], F32R)
            nc.sync.dma_start(out=molT_sb[:], in_=molT.ap().rearrange("(c p) q -> p c q", p=128))
            protT_sb = ph1.tile([128, ND, Lp], F32R)
            nc.sync.dma_start(out=protT_sb[:], in_=protT.ap().rearrange("(c p) q -> p c q", p=128))

            # ones columns of vhat
            for kc in range(NK):
                nc.vector.tensor_copy(vhat[:, kc, :, DK], ones_r[:, 0:H])

            # Q^T: lhsT = wq[di, hh-chunk], rhs = molT
            wq_sb = wslab.tile([128, ND, D], F32R, tag="w")
            nc.sync.dma_start(out=wq_sb[:], in_=wq.ap().rearrange("(c p) m -> p c m", p=128))
            for hh in range(NHH):
                q_ps = ph1ps.tile([128, Lm], F32, tag="qkps")
                for di in range(ND):
                    nc.tensor.matmul(q_ps[:], wq_sb[:, di, hh * 128:(hh + 1) * 128],
                                     molT_sb[:, di, :], start=(di == 0), stop=(di == ND - 1))
                if hh % 2 == 0:
                    nc.vector.tensor_copy(qTp[:, hh, :], q_ps[:])
                else:
                    nc.scalar.copy(qTp[:, hh, :], q_ps[:])

            # K^T
            wk_sb = wslab.tile([128, ND, D], F32R, tag="w")
            nc.sync.dma_start(out=wk_sb[:], in_=wk.ap().rearrange("(c p) m -> p c m", p=128))
            for hh in range(NHH):
                for kh in range(2):
                    k_ps = ph1ps.tile([128, 512], F32, tag="qkps")
                    for di in range(ND):
                        nc.tensor.matmul(k_ps[:], wk_sb[:, di, hh * 128:(hh + 1) * 128],
                                         protT_sb[:, di, kh * 512:(kh + 1) * 512],
                                         start=(di == 0), stop=(di == ND - 1))
                    if kh == 0:
                        nc.vector.tensor_copy(kTp[:, hh, kh * 512:(kh + 1) * 512], k_ps[:])
                    else:
                        nc.scalar.copy(kTp[:, hh, kh * 512:(kh + 1) * 512], k_ps[:])

            # V natural (into vhat), lhsT = protT chunk, rhs = wv cols
            wv_sb = wslab.tile([128, ND, D], F32R, tag="w")
            nc.sync.dma_start(out=wv_sb[:], in_=wv.ap().rearrange("(c p) m -> p c m", p=128))
            for kc in range(NK):
                for dh in range(2):
                    v_ps = ph1ps.tile([128, 384], F32, tag="vps")
                    for di in range(ND):
                        nc.tensor.matmul(v_ps[:], protT_sb[:, di, kc * 128:(kc + 1) * 128],
                                         wv_sb[:, di, dh * 384:(dh + 1) * 384],
                                         start=(di == 0), stop=(di == ND - 1))
                    dst = vhat[:, kc, dh * 6:(dh + 1) * 6, 0:DK]
                    src = v_ps[:].rearrange("p (h e) -> p h e", h=6)
                    if (kc + dh) % 2 == 0:
                        nc.vector.tensor_copy(dst, src)
                    else:
                        nc.scalar.copy(dst, src)

        # ---------------- Phase 2: attention per head ----------------
        with tc.tile_pool(name="et", bufs=2) as et_pool, \
             tc.tile_pool(name="en", bufs=2) as en_pool, \
             tc.tile_pool(name="pt", bufs=3) as pt_pool, \
             tc.tile_pool(name="dsc", bufs=2) as dsc_pool, \
             tc.tile_pool(name="aps", bufs=2, space="PSUM") as aps, \
             tc.tile_pool(name="ops", bufs=1, space="PSUM") as ops:
            for h in range(H):
                hh, par = h // 2, (h % 2) * 64
                # S^T -> exp with mask bias -> E^T (f32r)
                e_t = et_pool.tile([128, NK, Lm], F32R, tag="e_t")
                for kc in range(NK):
                    sT_ps = aps.tile([128, Lm], F32, tag="sT")
                    nc.tensor.matmul(sT_ps[:],
                                     kTp[par:par + DK, hh, kc * 128:(kc + 1) * 128],
                                     qTp[par:par + DK, hh, :], start=True, stop=True)
                    nc.scalar.activation(out=e_t[:, kc, :], in_=sT_ps[:], func=AF.Exp,
                                         bias=maskb_t[:, kc:kc + 1], scale=0.125)
                # PV: O^T (rows 0..63) and denominators D (row 64)
                oT_ps = ops.tile([DK + 1, Lm], F32, tag="oT", bufs=2)
                for kc in range(NK):
                    nc.tensor.matmul(oT_ps[:], vhat[:, kc, h, :], e_t[:, kc, :],
                                     start=(kc == 0), stop=(kc == NK - 1))
                # recip of denominators: row -> per-partition columns
                d_row = dsc_pool.tile([1, Lm], F32, tag="d_row")
                nc.scalar.copy(d_row[:], oT_ps[DK:DK + 1, :])
                rr = dsc_pool.tile([1, Lm], F32, tag="rr")
                nc.vector.reciprocal_approx_fast(rr[:], d_row[:])
                rd_ps = ops.tile([128, NQ], F32, tag="rd")
                for qc in range(NQ):
                    nc.tensor.matmul(rd_ps[:, qc:qc + 1], rr[0:1, qc * 128:(qc + 1) * 128],
                                     one11[:], start=True, stop=True)
                rd_nat = dsc_pool.tile([128, NQ], F32, tag="rd_nat")
                nc.vector.tensor_copy(rd_nat[:], rd_ps[:])
                # broadcast recip rows for O^T normalization
                rb_ps = ops.tile([DK, Lm], F32, tag="rb")
                nc.tensor.matmul(rb_ps[:], ones_f[0:1, 0:DK], rr[:], start=True, stop=True)
                rb_sb = dsc_pool.tile([DK, Lm], F32, tag="rb_sb")
                nc.scalar.copy(rb_sb[:], rb_ps[:])
                nc.vector.tensor_tensor(out=oT[par:par + DK, hh, :], in0=oT_ps[0:DK, :],
                                        in1=rb_sb[:], op=OP.mult)
                # S natural -> exp -> P = (E * recipD) * mask -> DMA out
                e_n = en_pool.tile([128, NQ, Lp], F32, tag="e_n")
                for qc in range(NQ):
                    sN_ps = aps.tile([128, Lp], F32, tag="sN", bufs=1)
                    for kh in range(2):
                        nc.tensor.matmul(sN_ps[:, kh * 512:(kh + 1) * 512],
                                         qTp[par:par + DK, hh, qc * 128:(qc + 1) * 128],
                                         kTp[par:par + DK, hh, kh * 512:(kh + 1) * 512],
                                         start=True, stop=True)
                    nc.scalar.activation(out=e_n[:, qc, :],
                                         in_=sN_ps[:], func=AF.Exp, scale=0.125)
                for qc in range(NQ):
                    p_t = pt_pool.tile([128, Lp], F32, tag="p_t")
                    nc.vector.scalar_tensor_tensor(
                        out=p_t[:], in0=e_n[:, qc, :], scalar=rd_nat[:, qc:qc + 1],
                        in1=mask_bc[:], op0=OP.mult, op1=OP.mult)
                    nc.sync.dma_start(out=attn_d[h, qc * 128:(qc + 1) * 128, :], in_=p_t[:])

        # ---------------- Phase 3: output projection + LN1 ----------------
        x1T = persist.tile([128, ND, Lm], F32R)
        x1n = persist.tile([128, NQ, D], F32)
        with tc.tile_pool(name="ph3", bufs=1) as ph3, \
             tc.tile_pool(name="ph3s", bufs=2) as ph3s, \
             tc.tile_pool(name="ph3ps", bufs=4, space="PSUM") as ph3ps:
            mol_sb = ph3.tile([128, NQ, D], F32)
            nc.sync.dma_start(out=mol_sb[:], in_=mol.ap().rearrange("(c p) d -> p c d", p=128))
            g1b = ph3.tile([128, D], F32)
            nc.sync.dma_start(out=g1b[:], in_=bcast(g1.ap(), D))
            b1b = ph3.tile([128, D], F32)
            nc.sync.dma_start(out=b1b[:], in_=bcast(bb1.ap(), D))
            wo_sb = ph3.tile([128, ND, D], F32R)
            nc.sync.dma_start(out=wo_sb[:], in_=wo.ap().rearrange("(c p) m -> p c m", p=128))

            aoT = ph3.tile([128, ND, Lm], F32)
            for do in range(ND):
                ao_ps = ph3ps.tile([128, Lm], F32, tag="ao")
                for di in range(ND):
                    nc.tensor.matmul(ao_ps[:], wo_sb[:, di, do * 128:(do + 1) * 128],
                                     oT[:, di, :], start=(di == 0), stop=(di == ND - 1))
                nc.scalar.activation(out=aoT[:, do, :], in_=ao_ps[:], func=AF.Identity,
                                     bias=bo_t[:, do:do + 1])
            # transpose attn_out^T back to natural, fuse residual add
            z = ph3.tile([128, NQ, D], F32)
            for qc in range(NQ):
                for dc in range(ND):
                    t_ps = ph3ps.tile([128, 128], F32, tag="tp")
                    nc.tensor.transpose(t_ps[:], aoT[:, dc, qc * 128:(qc + 1) * 128], ident[:])
                    nc.vector.tensor_tensor(out=z[:, qc, dc * 128:(dc + 1) * 128],
                                            in0=t_ps[:], in1=mol_sb[:, qc, dc * 128:(dc + 1) * 128],
                                            op=OP.add)
            # LN1 + affine
            for qc in range(NQ):
                st = ph3s.tile([128, 3, 6], F32, tag="st")
                for i in range(3):
                    nc.vector.bn_stats(out=st[:, i, :], in_=z[:, qc, i * 256:(i + 1) * 256])
                mv = ph3s.tile([128, 2], F32, tag="mv")
                nc.vector.bn_aggr(out=mv[:], in_=st[:])
                rstd = ph3s.tile([128, 1], F32, tag="rstd")
                nc.scalar.activation(out=rstd[:], in_=mv[:, 1:2], func=AF.Sqrt, bias=eps_t[:, 0:1])
                nc.vector.reciprocal(rstd[:], rstd[:])
                xc = ph3s.tile([128, D], F32, tag="xc")
                nc.vector.tensor_scalar(out=xc[:], in0=z[:, qc, :], scalar1=mv[:, 0:1],
                                        scalar2=rstd[:], op0=OP.subtract, op1=OP.mult)
                xg = ph3s.tile([128, D], F32, tag="xg")
                nc.vector.tensor_tensor(out=xg[:], in0=xc[:], in1=g1b[:], op=OP.mult)
                nc.vector.tensor_tensor(out=x1n[:, qc, :], in0=xg[:], in1=b1b[:], op=OP.add)
            # transpose x1 -> x1T
            for dc in range(ND):
                for qc in range(NQ):
                    t_ps = ph3ps.tile([128, 128], F32, tag="tp")
                    nc.tensor.transpose(t_ps[:], x1n[:, qc, dc * 128:(dc + 1) * 128], ident[:])
                    if (dc + qc) % 2 == 0:
                        nc.vector.tensor_copy(x1T[:, dc, qc * 128:(qc + 1) * 128], t_ps[:])
                    else:
                        nc.scalar.copy(x1T[:, dc, qc * 128:(qc + 1) * 128], t_ps[:])

        # ---------------- Phase 4: FFN + LN2 + output ----------------
        with tc.tile_pool(name="ph4", bufs=1) as ph4, \
             tc.tile_pool(name="ph4s", bufs=2) as ph4s:
            sT = ph4.tile([128, ND, Lm], F32)
            with tc.tile_pool(name="w1sl", bufs=2) as w1sl, \
                 tc.tile_pool(name="w2sl", bufs=2) as w2sl, \
                 tc.tile_pool(name="gsl", bufs=3) as gsl, \
                 tc.tile_pool(name="ff1ps", bufs=2, space="PSUM") as ff1ps, \
                 tc.tile_pool(name="ff2ps", bufs=1, space="PSUM") as ff2ps:
                ff2_ps = [ff2ps.tile([128, Lm], F32, tag=f"ff2_{i}", name=f"ff2_{i}") for i in range(ND)]
                for dd in range(NDD):
                    w1_sb = w1sl.tile([128, ND, 128], F32R, tag="w1")
                    nc.sync.dma_start(out=w1_sb[:], in_=w1p[dd].rearrange("(c p) m -> p c m", p=128))
                    ff1_ps = ff1ps.tile([128, Lm], F32, tag="ff1")
                    for di in range(ND):
                        nc.tensor.matmul(ff1_ps[:], w1_sb[:, di, :], x1T[:, di, :],
                                         start=(di == 0), stop=(di == ND - 1))
                    g_sb = gsl.tile([128, Lm], F32R, tag="g")
                    nc.scalar.activation(out=g_sb[:], in_=ff1_ps[:], func=AF.Gelu,
                                         bias=b1_t[:, dd:dd + 1])
                    w2_sb = w2sl.tile([128, D], F32R, tag="w2")
                    nc.sync.dma_start(out=w2_sb[:], in_=w2[dd * 128:(dd + 1) * 128, :])
                    for do in range(ND):
                        nc.tensor.matmul(ff2_ps[do][:], w2_sb[:, do * 128:(do + 1) * 128], g_sb[:],
                                         start=(dd == 0), stop=(dd == NDD - 1))
                # s^T = ff2^T + b2 + x1^T (evacuate before psum pools close)
                for do in range(ND):
                    nc.scalar.activation(out=sT[:, do, :], in_=ff2_ps[do][:], func=AF.Identity,
                                         bias=b2_t[:, do:do + 1])
                    nc.vector.tensor_tensor(out=sT[:, do, :], in0=sT[:, do, :],
                                            in1=x1T[:, do, :].bitcast(F32), op=OP.add)
            g2b = ph4.tile([128, D], F32)
            nc.sync.dma_start(out=g2b[:], in_=bcast(g2.ap(), D))
            b2b = ph4.tile([128, D], F32)
            nc.sync.dma_start(out=b2b[:], in_=bcast(bb2.ap(), D))
            s_n = ph4.tile([128, NQ, D], F32)
            tps = ctx.enter_context(tc.tile_pool(name="tps", bufs=2, space="PSUM"))
            for qc in range(NQ):
                for dc in range(ND):
                    t_ps = tps.tile([128, 128], F32, tag="tp2")
                    nc.tensor.transpose(t_ps[:], sT[:, dc, qc * 128:(qc + 1) * 128], ident[:])
                    if (dc + qc) % 2 == 0:
                        nc.vector.tensor_copy(s_n[:, qc, dc * 128:(dc + 1) * 128], t_ps[:])
                    else:
                        nc.scalar.copy(s_n[:, qc, dc * 128:(dc + 1) * 128], t_ps[:])
            # LN2 + affine + DMA out
            for qc in range(NQ):
                st2 = ph4s.tile([128, 3, 6], F32, tag="st2")
                for i in range(3):
                    nc.vector.bn_stats(out=st2[:, i, :], in_=s_n[:, qc, i * 256:(i + 1) * 256])
                mv2 = ph4s.tile([128, 2], F32, tag="mv2")
                nc.vector.bn_aggr(out=mv2[:], in_=st2[:])
                rstd2 = ph4s.tile([128, 1], F32, tag="rstd2")
                nc.scalar.activation(out=rstd2[:], in_=mv2[:, 1:2], func=AF.Sqrt, bias=eps_t[:, 0:1])
                nc.vector.reciprocal(rstd2[:], rstd2[:])
                oc = ph4s.tile([128, D], F32, tag="oc")
                nc.vector.tensor_scalar(out=oc[:], in0=s_n[:, qc, :], scalar1=mv2[:, 0:1],
                                        scalar2=rstd2[:], op0=OP.subtract, op1=OP.mult)
                og = ph4s.tile([128, D], F32, tag="og")
                nc.vector.tensor_tensor(out=og[:], in0=oc[:], in1=g2b[:], op=OP.mult)
                ob = ph4s.tile([128, D], F32, tag="ob")
                nc.vector.tensor_tensor(out=ob[:], in0=og[:], in1=b2b[:], op=OP.add)
                nc.sync.dma_start(out=out_d[qc * 128:(qc + 1) * 128, :], in_=ob[:])

    nc.compile()
    return nc


@lru_cache(maxsize=1)
def _get_nc():
    return _build()


def kernel(mol_h, prot_h, key_mask, Wq, Wk, Wv, Wo, bo,
           ln1_g, ln1_b, ln2_g, ln2_b, W1, b1, W2, b2):
    mol_h = np.ascontiguousarray(np.asarray(mol_h, dtype=np.float32))
    prot_h = np.ascontiguousarray(np.asarray(prot_h, dtype=np.float32))
    key_mask = np.asarray(key_mask).astype(bool)
    Wq = np.ascontiguousarray(np.asarray(Wq, dtype=np.float32))
    Wk = np.ascontiguousarray(np.asarray(Wk, dtype=np.float32))
    Wv = np.ascontiguousarray(np.asarray(Wv, dtype=np.float32))
    Wo = np.ascontiguousarray(np.asarray(Wo, dtype=np.float32))
    W1 = np.ascontiguousarray(np.asarray(W1, dtype=np.float32))
    W2 = np.ascontiguousarray(np.asarray(W2, dtype=np.float32))

    # all-masked guard (matches reference): unmask key 0 where every key masked
    all_masked = key_mask.all(axis=-1)
    safe_mask = key_mask.copy()
    safe_mask[all_masked, 0] = False

    # shared (per-core-identical) host-prepped weights
    w1p = np.ascontiguousarray(W1.reshape(D, NDD, 128).transpose(1, 0, 2))
    shared = {
        "wq": Wq, "wk": Wk, "wv": Wv, "wo": Wo, "w1p": w1p, "w2": W2,
        "bo_c": np.ascontiguousarray(np.asarray(bo, np.float32).reshape(ND, 128).T),
        "b1_c": np.ascontiguousarray(np.asarray(b1, np.float32).reshape(NDD, 128).T),
        "b2_c": np.ascontiguousarray(np.asarray(b2, np.float32).reshape(ND, 128).T),
        "g1": np.asarray(ln1_g, np.float32), "bb1": np.asarray(ln1_b, np.float32),
        "g2": np.asarray(ln2_g, np.float32), "bb2": np.asarray(ln2_b, np.float32),
    }
    in_maps = []
    for b_i in range(B):
        bias = np.where(safe_mask[b_i], np.float32(MASK_BIAS), np.float32(0.0))
        in_maps.append({
            "mol": mol_h[b_i],
            "molT": np.ascontiguousarray(mol_h[b_i].T),
            "protT": np.ascontiguousarray(prot_h[b_i].T),
            "maskb": np.ascontiguousarray(bias.reshape(NK, 128).T),
            "mask01": np.where(safe_mask[b_i], np.float32(0.0), np.float32(1.0)),
            **shared,
        })

    nc = _get_nc()
    res = run_bass_kernel_spmd(nc, in_maps, list(range(B)))
    x = np.stack([res.results[i]["out"] for i in range(B)])
    attn = np.stack([res.results[i]["attn"] for i in range(B)])
    return x, attn
